# revision 31
# baseline (speedup 1.0000x reference)
"""Trainium2 Bass kernel: 8-head transformer encoder layer (B=8, S=1024,
D=300, Dh=512, H=8), data-parallel over batch across 8 NeuronCores.

v3 schedule redesign vs v2:
  - ScalarE never swaps activation tables: Sqrt is replaced by Log+Exp
    (rstd = exp(-0.5*ln(var+eps))), so every ACT func used (exp, ln,
    identity, copy) lives in the one 'natural_log_exp_and_others' set.
  - all projections, score stats and v run BEFORE attention; attention
    is one continuous exp stream (0,0)..(3,1) with the AV matmuls
    chasing the exp chunks, and WO/LN1/FFN1 injected as PE filler
    between attention units so the PE never idles a HAM window.
  - x2 transposes are issued immediately after each wo_mm so the DMA
    queue finishes them long before FFN1 needs them.
  - wk/xt input DMAs interleaved so the first projection matmul can
    start after 2 transfers instead of 6.

Math identities (same as v2): beta_a and the score mean drop out by
softmax shift invariance; the 1/sqrt(D) score scale cancels inside the
score LayerNorm leaving gamma/sqrt(var_raw + D*eps); v is
ones-augmented so the AV matmul also yields the softmax denominators.
"""

import numpy as np
import ml_dtypes

import concourse.bass as bass
import concourse.tile as tile
from concourse import bacc, mybir
from concourse.bass_utils import run_bass_kernel_spmd
from concourse.masks import make_identity

F32 = mybir.dt.float32
F32R = mybir.dt.float32r
BF = mybir.dt.bfloat16
I32 = mybir.dt.int32
AF = mybir.ActivationFunctionType
ALU = mybir.AluOpType

B, S, D, DH, H, DHD = 8, 1024, 300, 512, 8, 64
DF = 4 * D  # 1200
EPS = 1e-8
NCORES = 8

DP, DFP = 384, 1280  # D/DF zero-padded to 128 multiples
J_CHUNKS = [(0, 128), (128, 128), (256, 128)]
N_ST = S // 128  # 8 s-tiles
N_SH = S // 512  # 2 s-halves
LCORR = float(D) / float(D - 1)

TRACE = False
_cache = {}
_last_results = None
PHASE_MARKS = []


def _build_nc(dbg=False):
    PHASE_MARKS.clear()
    nc = bacc.Bacc("TRN2", debug=False)

    def mark(label):
        PHASE_MARKS.append((label, int(nc.next_id())))

    xd = nc.dram_tensor("x", [S, D], F32, kind="ExternalInput").ap()
    xtd = nc.dram_tensor("xt", [DP, S], BF, kind="ExternalInput").ap()
    wqd = nc.dram_tensor("wq", [DP, DH], BF, kind="ExternalInput").ap()
    wkd = nc.dram_tensor("wk", [DP, DH], BF, kind="ExternalInput").ap()
    wvd = nc.dram_tensor("wv", [DP, DH], BF, kind="ExternalInput").ap()
    wod = nc.dram_tensor("wo", [DH, D], BF, kind="ExternalInput").ap()
    w1d = nc.dram_tensor("w1", [DP, DFP], BF, kind="ExternalInput").ap()
    w2d = nc.dram_tensor("w2", [DFP, D], BF, kind="ExternalInput").ap()
    fb1d = nc.dram_tensor("fb1", [DFP, 1], F32, kind="ExternalInput").ap()
    fb2d = nc.dram_tensor("fb2", [D], F32, kind="ExternalInput").ap()
    gad = nc.dram_tensor("ga", [H, 1], F32, kind="ExternalInput").ap()
    lnd = nc.dram_tensor("ln", [4, 1], F32, kind="ExternalInput").ap()
    seld = nc.dram_tensor("sel", [4, 2, 128], BF, kind="ExternalInput").ap()
    outd = nc.dram_tensor("out", [S, D], F32, kind="ExternalOutput").ap()

    with tile.TileContext(nc) as tc:
        with (
            tc.tile_pool(name="wts", bufs=1) as wts,
            tc.tile_pool(name="work", bufs=1) as work,
            tc.tile_pool(name="sm", bufs=8) as sm,
            tc.tile_pool(name="ps", bufs=1, space="PSUM") as ps,
        ):
            # ---------------- constants ----------------
            ident_bf = wts.tile([128, 128], BF, tag="ident")
            make_identity(nc, ident_bf)

            ones_bf = wts.tile([128, 1], BF, tag="onesb")
            nc.vector.memset(ones_bf, 1.0)

            eps_a = wts.tile([128, 1], F32, tag="eps_a")  # D*EPS (score LN)
            nc.vector.memset(eps_a, D * EPS)
            eps_l = wts.tile([128, 1], F32, tag="eps_l")  # EPS (x LNs)
            nc.vector.memset(eps_l, EPS)
            # seed for the DVE bit-trick rsqrt (keeps Sqrt off ScalarE so
            # its table never leaves the exp set mid-attention)
            magic_i = wts.tile([128, 1], I32, tag="magic")
            nc.vector.memset(magic_i, 0x5F3759DF)

            # ---------------- input loads (bf16) ----------------
            # wk/xt interleaved: the first proj matmul only needs wk0+xt0
            wk_sb = []
            xt_sb = []
            for jc, (j0, jn) in enumerate(J_CHUNKS):
                w = wts.tile([128, DH], BF, tag=f"wk{jc}")
                nc.sync.dma_start(out=w, in_=wkd[j0 : j0 + jn, :])
                wk_sb.append(w)
                t = wts.tile([128, S], BF, tag=f"xt{jc}", name=f"xt{jc}")
                nc.sync.dma_start(out=t[:, 0:512], in_=xtd[j0 : j0 + jn, 0:512])
                xt_sb.append(t)
            for jc, (j0, jn) in enumerate(J_CHUNKS):
                nc.sync.dma_start(
                    out=xt_sb[jc][:, 512:1024], in_=xtd[j0 : j0 + jn, 512:1024])

            def chunked_load(src, width, tag):
                tiles = []
                for jc, (j0, jn) in enumerate(J_CHUNKS):
                    t = wts.tile([128, width], BF, tag=f"{tag}{jc}")
                    nc.sync.dma_start(out=t, in_=src[j0 : j0 + jn, :])
                    tiles.append(t)
                return tiles

            wq_sb = chunked_load(wqd, DH, "wq")
            wv_sb = chunked_load(wvd, DH, "wv")
            w1_sb = chunked_load(w1d, DFP, "w1")

            wo_sb = []
            for it in range(4):
                t = wts.tile([128, D], BF, tag=f"wo{it}")
                nc.sync.dma_start(out=t, in_=wod[it * 128 : (it + 1) * 128, :])
                wo_sb.append(t)
            w2_sb = []
            for mt in range(10):
                t = wts.tile([128, D], BF, tag=f"w2_{mt}")
                nc.sync.dma_start(out=t, in_=w2d[mt * 128 : (mt + 1) * 128, :])
                w2_sb.append(t)

            # x natural: [128, 8, 300] (partition = s % 128) for residuals
            x_sb = wts.tile([128, N_ST, D], F32, tag="x")
            nc.sync.dma_start(out=x_sb, in_=xd.rearrange("(n p) d -> p n d", p=128))

            # small constants AFTER the big loads: their many-descriptor
            # broadcast DMAs must not delay the first projection matmuls
            # sel4[k, p, m] = 1 iff k == 2p + (m >= 64): picks the head-pair
            # rows out of a 4-head c8 batch for the bcast matmul
            # small constants go on the GpSimd DMA queue: their
            # many-descriptor broadcast transfers must not occupy the Sync
            # queue that the stats srow collection waits behind
            sel4 = wts.tile([4, 2, 128], BF, tag="sel")
            nc.gpsimd.dma_start(out=sel4, in_=seld)
            ga_h = []
            for b2_ in range(2):
                t = wts.tile([4, 1], F32, tag=f"ga{b2_}")
                nc.gpsimd.dma_start(out=t, in_=gad[b2_ * 4 : b2_ * 4 + 4, :])
                ga_h.append(t)

            def bcast_load(src_ap, shape, tag):
                t = wts.tile(shape, F32, tag=tag)
                nc.gpsimd.dma_start(out=t, in_=src_ap.to_broadcast(shape))
                return t

            g1_bc = bcast_load(lnd[0:1, :], [128, 1], "g1")
            b1_bc = bcast_load(lnd[1:2, :], [128, 1], "b1")
            g2_bc = bcast_load(lnd[2:3, :], [128, 1], "g2")
            b2_bc = bcast_load(lnd[3:4, :], [128, 1], "b2")
            fb2_bc = wts.tile([128, D], F32, tag="fb2")
            nc.gpsimd.dma_start(
                out=fb2_bc,
                in_=bass.AP(tensor=fb2d.tensor, offset=fb2d.offset,
                            ap=[[0, 128]] + list(fb2d.ap)),
            )
            fb1_sb = []
            for mt in range(10):
                t = wts.tile([128, 1], F32, tag=f"fb1_{mt}")
                nc.gpsimd.dma_start(out=t, in_=fb1d[mt * 128 : (mt + 1) * 128, :])
                fb1_sb.append(t)

            # ---------------- persistent activations ----------------
            NBIG = 22  # h1T recycles the (dead-by-then) qT slots
            qT = [work.tile([128, S], BF, tag="big", bufs=NBIG, name=f"qT{i}")
                  for i in range(4)]
            kT = [work.tile([128, S], BF, tag="big", bufs=NBIG, name=f"kT{i}")
                  for i in range(4)]
            qs = [work.tile([128, S], BF, tag="big", bufs=NBIG, name=f"qs{i}")
                  for i in range(4)]
            aT = [work.tile([128, S], BF, tag="big", bufs=NBIG, name=f"aT{i}")
                  for i in range(4)]
            h1T = [work.tile([128, S], BF, tag="big", bufs=NBIG,
                             name=f"h1T{i}") for i in range(10)]
            kn_sb = work.tile([128, N_ST, DH], BF, tag="kn", name="kn")
            v_sb = work.tile([128, N_ST, H, DHD], BF, tag="v", name="v")
            x2T = work.tile([128, 3, S], BF, tag="x2T", name="x2T")
            x2b = work.tile([128, N_ST, DP], BF, tag="x2b", name="x2b")
            nc.vector.memset(x2b[:, :, D:DP], 0.0)
            # score stats, split into two 4-head batches (heads 0-3 / 4-7)
            s1_t = [wts.tile([4, S], F32, tag=f"s1_{i}", name=f"s1_{i}") for i in range(2)]
            s2_t = [wts.tile([4, S], F32, tag=f"s2_{i}", name=f"s2_{i}") for i in range(2)]
            c8_t = [wts.tile([4, S], BF, tag=f"c8_{i}", name=f"c8_{i}") for i in range(2)]

            # ---------------- phase 1: k projection + G + ksum ----------------
            def proj_unit(dst, w_sb, dt, copy_eng):
                pp = ps.tile([128, 2, 512], F32, tag="e", bufs=2, name="pp")
                for sh in range(N_SH):
                    for jc, (j0, jn) in enumerate(J_CHUNKS):
                        nc.tensor.matmul(
                            pp[:, sh, :],
                            lhsT=w_sb[jc][:, dt * 128 : (dt + 1) * 128],
                            rhs=xt_sb[jc][:, sh * 512 : (sh + 1) * 512],
                            start=(jc == 0),
                            stop=(jc == 2),
                            skip_group_check=True,
                        )
                copy_eng(out=dst[dt], in_=pp.rearrange("p a b -> p (a b)"))

            def act_copy(out, in_):
                nc.scalar.copy(out=out, in_=in_)

            def vec_copy(out, in_):
                nc.vector.tensor_copy(out=out, in_=in_)

            mark("proj_k")
            ksum_bf = [None] * 4

            def ksum_unit(i):
                kf = sm.tile([128, 1], F32, tag="ksf", bufs=4, name=f"ksf{i}")
                nc.vector.reduce_sum(out=kf, in_=kT[i], axis=mybir.AxisListType.X)
                kb = wts.tile([128, 1], BF, tag=f"ksb{i}")
                nc.vector.tensor_copy(out=kb, in_=kf)
                ksum_bf[i] = kb

            for dt in range(4):
                proj_unit(kT, wk_sb, dt, act_copy)
            for i in range(4):
                ksum_unit(i)

            # kn projection
            for st in range(N_ST):
                pp = ps.tile([128, 512], F32, tag="f", bufs=2, name="knp")
                for jc, (j0, jn) in enumerate(J_CHUNKS):
                    nc.tensor.matmul(
                        pp,
                        lhsT=xt_sb[jc][:, st * 128 : (st + 1) * 128],
                        rhs=wk_sb[jc],
                        start=(jc == 0),
                        stop=(jc == 2),
                    )
                nc.vector.tensor_copy(out=kn_sb[:, st, :], in_=pp)

            # G via a streaming half-Gram: for each 128-row block of kn
            # (2 heads), one N=512 accumulation over all s-tiles — large
            # pipelined matmuls instead of 128 tiny N=64 ones; the
            # per-head diagonal blocks are then sliced out on the DVE
            # (both partition halves, so stats j=0/1 read aligned rows).
            G_sb = wts.tile([128, H, DHD], BF, tag="gsb")

            def g_unit(dt):
                gp = ps.tile([128, 512], F32, tag="av", bufs=2, name="gp")
                for st in range(N_ST):
                    nc.tensor.matmul(
                        gp,
                        lhsT=kn_sb[:, st, dt * 128 : (dt + 1) * 128],
                        rhs=kn_sb[:, st, :],
                        start=(st == 0),
                        stop=(st == N_ST - 1),
                    )
                for j in range(2):
                    h = 2 * dt + j
                    for hp in (0, DHD):
                        nc.vector.tensor_copy(
                            out=G_sb[hp : hp + DHD, h, :],
                            in_=gp[j * 64 : (j + 1) * 64,
                                   h * 64 : (h + 1) * 64])

            for dt in range(4):
                g_unit(dt)

            # ---------------- phase 2: row-form score stats ----------------
            def stats_pair(hq):
                y_ps = ps.tile([128, S], F32, tag="e", bufs=2, name="y")
                z_t = work.tile([128, S], BF, tag="z", bufs=2, name="z")
                srows = []
                # all independent PE work first (y both heads, e-rows), then
                # one merged z multiply, then the z-rows that depend on it
                for j in range(2):
                    h, hp = 2 * hq + j, j * 64
                    for sh in range(N_SH):
                        nc.tensor.matmul(
                            y_ps[hp : hp + 64, sh * 512 : (sh + 1) * 512],
                            lhsT=G_sb[hp : hp + 64, h, :],
                            rhs=qT[hq][hp : hp + 64, sh * 512 : (sh + 1) * 512],
                            start=True, stop=True, skip_group_check=True,
                        )
                def erows(j):
                    hp = j * 64
                    srow = ps.tile([33, S], F32, tag="e", bufs=2,
                                   name=f"srow{j}")
                    srows.append(srow)
                    for sh in range(N_SH):
                        nc.tensor.matmul(
                            srow[0:1, sh * 512 : (sh + 1) * 512],
                            lhsT=ksum_bf[hq][hp : hp + 64, :],
                            rhs=qT[hq][hp : hp + 64, sh * 512 : (sh + 1) * 512],
                            start=True, stop=True, skip_group_check=True,
                        )

                erows(0)
                # z before the second srow alloc: that alloc retires y_ps
                # in the 2-deep "e" ring
                nc.vector.tensor_tensor(out=z_t, in0=qT[hq], in1=y_ps,
                                        op=mybir.AluOpType.mult)
                erows(1)
                for j in range(2):
                    hp = j * 64
                    for sh in range(N_SH):
                        nc.tensor.matmul(
                            srows[j][32:33, sh * 512 : (sh + 1) * 512],
                            lhsT=ones_bf[hp : hp + 64, :],
                            rhs=z_t[hp : hp + 64, sh * 512 : (sh + 1) * 512],
                            start=True, stop=True, skip_group_check=True,
                        )
                # srow drain on DVE, collection DMAs on the Vector queue:
                # the Sync queue is still busy with input loads and would
                # delay the stats critical path by ~20us
                for j in range(2):
                    srow_sb = work.tile([33, S], F32, tag="srow", bufs=2,
                                        name="srow_sb")
                    nc.scalar.copy(out=srow_sb, in_=srows[j])
                    hb, hr = hq // 2, 2 * (hq % 2) + j
                    nc.scalar.dma_start(out=s1_t[hb][hr : hr + 1, :],
                                        in_=srow_sb[0:1, :])
                    nc.scalar.dma_start(out=s2_t[hb][hr : hr + 1, :],
                                        in_=srow_sb[32:33, :])

            # ---------------- v projection ----------------
            # v units run inside the attention exp stream, so their psum
            # drains go through the Vector engine, never ScalarE
            def v_unit(stp):
                for st in (stp * 2, stp * 2 + 1):
                    pp = ps.tile([128, 512], F32, tag="f", bufs=2, name="vp")
                    for jc, (j0, jn) in enumerate(J_CHUNKS):
                        nc.tensor.matmul(
                            pp,
                            lhsT=xt_sb[jc][:, st * 128 : (st + 1) * 128],
                            rhs=wv_sb[jc],
                            start=(jc == 0),
                            stop=(jc == 2),
                        )
                    nc.vector.tensor_copy(
                        out=v_sb[:, st, :, 0:DHD],
                        in_=pp.rearrange("p (h d) -> p h d", h=H))

            # stats math for one 4-head batch -> c8_t[hb]. ACT Sqrt is fine
            # here: both stats_math calls run before the first attention
            # Exp, so the sqrt->exp table swap happens exactly once.
            def stats_math(hb):
                sq = sm.tile([4, S], F32, tag="stat", bufs=2, name="sq")
                nc.vector.tensor_tensor(out=sq, in0=s1_t[hb], in1=s1_t[hb],
                                        op=ALU.mult)
                m2 = sm.tile([4, S], F32, tag="stat", bufs=2, name="m2")
                nc.vector.scalar_tensor_tensor(
                    out=m2, in0=sq, scalar=-1.0 / S, in1=s2_t[hb],
                    op0=ALU.mult, op1=ALU.add)
                sd = sm.tile([4, S], F32, tag="stat", bufs=2, name="sd")
                nc.scalar.activation(out=sd, in_=m2, func=AF.Sqrt,
                                     bias=eps_a[0:4, :], scale=1.0 / (S - 1))
                rinv = sm.tile([4, S], F32, tag="stat", bufs=2, name="rinv")
                nc.vector.reciprocal_approx_fast(out=rinv, in_=sd)
                nc.vector.tensor_scalar_mul(c8_t[hb], rinv, ga_h[hb])

            # qs = qT * bcast(c8row)
            def qs_unit(hq):
                bc_ps = ps.tile([128, S], F32, tag="e", bufs=2, name="bc")
                for sh in range(N_SH):
                    nc.tensor.matmul(
                        bc_ps[:, sh * 512 : (sh + 1) * 512],
                        lhsT=sel4[:, hq % 2, :],
                        rhs=c8_t[hq // 2][:, sh * 512 : (sh + 1) * 512],
                        start=True, stop=True, skip_group_check=True,
                    )
                nc.vector.tensor_tensor(out=qs[hq], in0=qT[hq], in1=bc_ps,
                                        op=mybir.AluOpType.mult)

            # pre-attention: projections + stats — interleaved so the
            # stats DVE/ACT chains hide under the qT projection PE work.
            # Both stats_math sqrt calls land before the first Exp, so
            # ScalarE swaps tables exactly once.
            mark("pre_stats")
            proj_unit(qT, wq_sb, 0, vec_copy)
            stats_pair(0)
            proj_unit(qT, wq_sb, 1, vec_copy)
            stats_pair(1)
            stats_math(0)
            qs_unit(0)
            qs_unit(1)
            proj_unit(qT, wq_sb, 2, vec_copy)
            stats_pair(2)
            proj_unit(qT, wq_sb, 3, vec_copy)
            stats_pair(3)
            stats_math(1)
            qs_unit(2)
            qs_unit(3)

            # ---------------- phase 3: attention building blocks ----------
            # One unit = 8 score chunks, one per t-tile, covering BOTH
            # heads via 2x2 quadrant tile packing (K=64 row groups x M=64
            # col groups) — the full PE array works on every chunk. The
            # 3-deep e2 psum ring gives the in-order PE stream ~2 chunks
            # of lookahead over the exp pace; the previous unit's
            # col-packed AV + r matmuls fill the gaps.
            UNITS = [(0, 0), (1, 0), (2, 0), (3, 0),
                     (0, 1), (1, 1), (2, 1), (3, 1)]

            def av_mms(prev, tj):
                (hq, sh), pT, av2, r2 = prev
                # both heads' AV concurrently: col groups 0-63 / 64-127
                for j in range(2):
                    nc.tensor.matmul(
                        av2[j * 64 : (j + 1) * 64, :],
                        lhsT=v_sb[:, tj, 2 * hq + j, :],
                        rhs=pT[:, tj, j, :],
                        start=(tj == 0),
                        stop=(tj == N_ST - 1),
                        skip_group_check=True,
                        tile_position=(0, j * 64),
                    )
                # both heads' softmax denominators, col groups 0-31 / 32-63
                for j in range(2):
                    nc.tensor.matmul(
                        r2[j * 32 : j * 32 + 1, :],
                        lhsT=ones_bf,
                        rhs=pT[:, tj, j, :],
                        start=(tj == 0),
                        stop=(tj == N_ST - 1),
                        skip_group_check=True,
                        tile_position=(0, j * 32),
                    )

            def av_epilogue(prev):
                (hq, sh), pT, av2, r2 = prev
                for j in range(2):
                    hp = j * 64
                    # r to partition 0 first: the custom-DVE reciprocal
                    # mis-reads unless in/out start at partition 0
                    rrow = sm.tile([1, 512], F32, tag="rrow", bufs=2)
                    nc.vector.tensor_copy(out=rrow,
                                          in_=r2[j * 32 : j * 32 + 1, :])
                    nc.vector.reciprocal_approx_fast(out=rrow, in_=rrow)
                    # broadcast 1/r across the 64 head dims on the idle
                    # GPSIMD engine (partition 0 -> all partitions)
                    rbc_sb = sm.tile([DHD, 512], F32, tag="rbcs", bufs=2)
                    nc.gpsimd.partition_broadcast(rbc_sb, rrow)
                    nc.vector.tensor_tensor(
                        out=aT[hq][hp : hp + 64, sh * 512 : (sh + 1) * 512],
                        in0=av2[hp : hp + 64, :], in1=rbc_sb,
                        op=ALU.mult,
                    )

            def attn_unit(k, prev, fillers):
                """Emit unit k's score/exp chunks, interleaved with unit
                k-1's AV matmuls and the filler thunks for this unit.
                Returns (unit, pT, av2, r2) to pass as next unit's prev."""
                hq, sh = UNITS[k]
                pT = work.tile([128, N_ST, 2, 512], BF, tag="pt", bufs=2,
                               name="pT")
                if prev is not None:
                    av2_p = ps.tile([128, 512], F32, tag="av", bufs=2,
                                    name="av2")
                    r2_p = ps.tile([33, 512], F32, tag="av", bufs=2,
                                   name="r2")
                    prev = (prev[0], prev[1], av2_p, r2_p)
                for tq in range(N_ST):
                    e2 = ps.tile([128, 2, 512], F32, tag="e", bufs=2,
                                 name="e2")
                    for j in range(2):
                        for ch in range(2):
                            nc.tensor.matmul(
                                e2[ch * 64 : (ch + 1) * 64, j, :],
                                lhsT=kT[hq][j * 64 : (j + 1) * 64,
                                            tq * 128 + ch * 64 :
                                            tq * 128 + (ch + 1) * 64],
                                rhs=qs[hq][j * 64 : (j + 1) * 64,
                                           sh * 512 : (sh + 1) * 512],
                                start=True, stop=True, skip_group_check=True,
                                tile_position=(j * 64, ch * 64),
                            )
                    nc.scalar.activation(out=pT[:, tq, :, :], in_=e2,
                                         func=AF.Exp)
                    if prev is not None:
                        av_mms(prev, tq)
                    for f in fillers.get(tq, ()):
                        f()
                if prev is not None:
                    av_epilogue(prev)
                return ((hq, sh), pT, None, None)

            # ---------------- LN helper ----------------
            # rstd via the DVE bit-trick (Quake seed + 2 Newton steps,
            # ~5e-6 rel err): keeps Sqrt off ScalarE entirely, so the ACT
            # table stays on the exp set for the whole attention stream.
            def ln_scalars(xr, g_bc, b_bc):
                stats = sm.tile([128, 6], F32, tag="lst", bufs=4)
                nc.vector.bn_stats(out=stats, in_=xr)
                mv = sm.tile([128, 2], F32, tag="lmv", bufs=4)
                nc.vector.bn_aggr(out=mv, in_=stats)
                vb = sm.tile([128, 1], F32, tag="lvb", bufs=4)
                nc.vector.tensor_scalar(out=vb, in0=mv[:, 1:2],
                                        scalar1=LCORR, scalar2=EPS,
                                        op0=ALU.mult, op1=ALU.add)
                ti = sm.tile([128, 1], I32, tag="lti", bufs=4)
                nc.vector.tensor_scalar(out=ti, in0=vb.bitcast(I32),
                                        scalar1=1, scalar2=None,
                                        op0=ALU.logical_shift_right)
                y0i = sm.tile([128, 1], I32, tag="ly0", bufs=4)
                nc.vector.tensor_tensor(out=y0i, in0=magic_i, in1=ti,
                                        op=ALU.subtract)
                y = y0i.bitcast(F32)
                for it_ in range(2):
                    a_ = sm.tile([128, 1], F32, tag="lnr", bufs=8)
                    nc.vector.tensor_tensor(out=a_, in0=y, in1=y, op=ALU.mult)
                    b_ = sm.tile([128, 1], F32, tag="lnr", bufs=8)
                    nc.vector.tensor_tensor(out=b_, in0=a_, in1=vb,
                                            op=ALU.mult)
                    c_ = sm.tile([128, 1], F32, tag="lnr", bufs=8)
                    nc.vector.tensor_scalar(out=c_, in0=b_, scalar1=-0.5,
                                            scalar2=1.5, op0=ALU.mult,
                                            op1=ALU.add)
                    y2 = sm.tile([128, 1], F32, tag="lnr", bufs=8)
                    nc.vector.tensor_tensor(out=y2, in0=y, in1=c_,
                                            op=ALU.mult)
                    y = y2
                grstd = sm.tile([128, 1], F32, tag="lgr", bufs=4)
                nc.vector.tensor_mul(grstd, y, g_bc)
                nb = sm.tile([128, 1], F32, tag="lnb", bufs=4)
                nc.vector.tensor_mul(nb, mv[:, 0:1], grstd)
                bias2 = sm.tile([128, 1], F32, tag="lb2", bufs=4)
                nc.vector.tensor_sub(bias2, b_bc, nb)
                return grstd, bias2

            # ---------------- WO + LN1 + x2 transpose ----------------
            def wo_mm(st):
                x1 = ps.tile([128, D], F32, tag="f", bufs=2, name="x1")
                for it in range(4):
                    nc.tensor.matmul(
                        x1,
                        lhsT=aT[it][:, st * 128 : (st + 1) * 128],
                        rhs=wo_sb[it],
                        start=(it == 0),
                        stop=(it == 3),
                    )
                xr = sm.tile([128, D], F32, tag="xr", bufs=2, name="xr1")
                nc.vector.tensor_add(xr, x1, x_sb[:, st, :])
                grstd, bias2 = ln_scalars(xr, g1_bc, b1_bc)
                ln_final(x2b[:, st, 0:D], xr, grstd, bias2, 0)

            def wo_tr(st, tail=False):
                # in the tail, split across the Sync and Scalar DMA queues
                # (ScalarE is idle once the exp stream is done) so the 3
                # transposes of one s-tile don't fully serialize
                for jc in range(3):
                    eng = nc.scalar if (tail and jc == 2) else nc.sync
                    eng.dma_start_transpose(
                        out=x2T[:, jc, st * 128 : (st + 1) * 128],
                        in_=x2b[:, st, jc * 128 : (jc + 1) * 128])

            def wo_unit(st, tail=False):
                wo_mm(st)
                fx2_pre(st)
                wo_tr(st, tail)

            # ---------------- FFN ----------------
            # ffn1 is split into s-tile chunks (N=128) so each chunk only
            # needs its own s-tile's x2T transposes, not the whole half
            def ffn1_unit(mt, sh):
                h1 = ps.tile([128, 512], F32, tag="f", bufs=2, name="h1")
                for st4 in range(4):
                    st = sh * 4 + st4
                    for jc in range(3):
                        nc.tensor.matmul(
                            h1[:, st4 * 128 : (st4 + 1) * 128],
                            lhsT=w1_sb[jc][:, mt * 128 : (mt + 1) * 128],
                            rhs=x2T[:, jc, st * 128 : (st + 1) * 128],
                            start=(jc == 0),
                            stop=(jc == 2),
                            skip_group_check=True,
                        )
                nc.vector.tensor_scalar(
                    out=h1T[mt][:, sh * 512 : (sh + 1) * 512],
                    in0=h1, scalar1=fb1_sb[mt], scalar2=0.0,
                    op0=ALU.add, op1=ALU.max)

            def ln_final(out_ap, xr, grstd, bias2, st):
                if st % 2 == 0:
                    nc.vector.tensor_scalar(
                        out=out_ap, in0=xr, scalar1=grstd, scalar2=bias2,
                        op0=mybir.AluOpType.mult, op1=mybir.AluOpType.add)
                else:
                    nc.scalar.activation(out=out_ap, in_=xr, func=AF.Identity,
                                         bias=bias2, scale=grstd)

            fx2 = work.tile([128, N_ST, D], BF, tag="fx2", name="fx2")

            def fx2_pre(st):
                nc.gpsimd.tensor_tensor(
                    out=fx2[:, st, :], in0=fb2_bc, in1=x2b[:, st, 0:D],
                    op=mybir.AluOpType.add)

            def ffn2_ln(st, h2):
                xr = sm.tile([128, D], F32, tag="xr", bufs=2, name="xr2")
                nc.vector.tensor_add(xr, h2, fx2[:, st, :])
                grstd, bias2 = ln_scalars(xr, g2_bc, b2_bc)
                o = sm.tile([128, D], F32, tag="o", bufs=2, name="o")
                ln_final(o, xr, grstd, bias2, st)
                nc.sync.dma_start(
                    out=outd[st * 128 : (st + 1) * 128, :], in_=o)

            def ffn2_unit(st):
                h2 = ps.tile([128, D], F32, tag="f", bufs=2, name="h2")
                for mt in range(10):
                    nc.tensor.matmul(
                        h2,
                        lhsT=h1T[mt][:, st * 128 : (st + 1) * 128],
                        rhs=w2_sb[mt],
                        start=(mt == 0),
                        stop=(mt == 9),
                    )
                ffn2_ln(st, h2)

            # ---------------- attention + filler schedule ----------------
            # exp stream: (0,0)(1,0)(2,0)(3,0)(0,1)(1,1)(2,1)(3,1).
            # fillers per unit: v while unit 0's exps run (AV(0,0) needs
            # it in unit 1); WO after AV(3,0) retires (end of unit 4);
            # ffn1-s0 once the first transposes have landed.
            mark("attn")

            def vf(st):
                return lambda: v_unit_st(st)

            def v_unit_st(st):
                pp = ps.tile([128, 512], F32, tag="f", bufs=2, name="vp")
                for jc, (j0, jn) in enumerate(J_CHUNKS):
                    nc.tensor.matmul(
                        pp,
                        lhsT=xt_sb[jc][:, st * 128 : (st + 1) * 128],
                        rhs=wv_sb[jc],
                        start=(jc == 0),
                        stop=(jc == 2),
                    )
                nc.vector.tensor_copy(
                    out=v_sb[:, st, :, 0:DHD],
                    in_=pp.rearrange("p (h d) -> p h d", h=H))

            # v slices ride in unit 0 (no prev-AV there, so the av psum
            # ring is free); all other units run pure scores+AV so the
            # exp stream never waits on filler matmuls
            u = attn_unit(0, None, {tq: [vf(tq)] for tq in range(8)})
            for k in range(1, 5):
                u = attn_unit(k, u, {})
            # aT s-half 0 completed with AV(3,0)'s epilogue (end of unit
            # 4): WO and then FFN1-s0 become legal filler work
            u = attn_unit(5, u, {1: [lambda: wo_unit(0)],
                                 5: [lambda: wo_unit(1)]})
            u = attn_unit(6, u, {1: [lambda: wo_unit(2)],
                                 5: [lambda: wo_unit(3)]})
            u = attn_unit(7, u, {})

            # ---------------- post-attention tail ----------------
            # AV(3,1)'s matmuls chase its last exps, then the whole
            # WO/FFN pipeline runs PE-dense: wo launches its transposes
            # early so ffn1's split s-tile chunks are never starved.
            mark("tail")
            av2_p = ps.tile([128, 512], F32, tag="av", bufs=2, name="av2")
            r2_p = ps.tile([33, 512], F32, tag="av", bufs=2, name="r2")
            u = (u[0], u[1], av2_p, r2_p)
            av_mms(u, 0)
            ffn1_unit(0, 0)
            av_mms(u, 1)
            ffn1_unit(1, 0)
            av_mms(u, 2)
            ffn1_unit(2, 0)
            av_mms(u, 3)
            ffn1_unit(3, 0)
            av_mms(u, 4)
            ffn1_unit(4, 0)
            av_mms(u, 5)
            ffn1_unit(5, 0)
            av_mms(u, 6)
            ffn1_unit(6, 0)
            av_mms(u, 7)
            av_epilogue(u)
            wo_unit(4, tail=True)
            ffn1_unit(7, 0)
            wo_unit(5, tail=True)
            ffn1_unit(8, 0)
            wo_unit(6, tail=True)
            ffn1_unit(9, 0)
            wo_unit(7, tail=True)
            mark("ffn2_s0")
            for st in range(4):
                ffn2_unit(st)
            mark("ffn_s1")
            for mt in range(10):
                ffn1_unit(mt, 1)
            for st in range(4, 8):
                ffn2_unit(st)

    nc.compile()
    return nc


def _get_nc():
    if "nc" not in _cache:
        _cache["nc"] = _build_nc()
    return _cache["nc"]


def kernel(x, WQ, WK, WV, WO, W1, b1, W2, b2, gamma_a, beta_a,
           gamma1, beta1, gamma2, beta2):
    global _last_results
    f = np.float32
    bf = ml_dtypes.bfloat16
    x = np.asarray(x, f)

    def perm(W):
        # head h -> contiguous rows [h*64, (h+1)*64)
        return np.asarray(W, f).reshape(DHD, H, D).transpose(1, 0, 2).reshape(DH, D)

    def padr(a, rows, cols=None):
        out = np.zeros((rows, cols or a.shape[1]), f)
        out[: a.shape[0], : a.shape[1]] = a
        return out

    wq_t = padr(perm(WQ).T, DP).astype(bf)
    wk_t = padr(perm(WK).T, DP).astype(bf)
    wv_t = padr(perm(WV).T, DP).astype(bf)
    wo = np.ascontiguousarray(np.asarray(WO, f)).astype(bf)
    w1 = padr(np.asarray(W1, f), DP, DFP).astype(bf)
    w2 = padr(np.asarray(W2, f), DFP).astype(bf)
    fb1 = np.zeros((DFP, 1), f)
    fb1[:DF, 0] = np.asarray(b1, f)
    fb2 = np.ascontiguousarray(np.asarray(b2, f))
    ga = np.ascontiguousarray(np.asarray(gamma_a, f).reshape(H, 1))
    ln = np.array(
        [np.asarray(gamma1, f), np.asarray(beta1, f),
         np.asarray(gamma2, f), np.asarray(beta2, f)], f
    ).reshape(4, 1)

    sel_np = np.zeros((4, 2, 128), f)
    for p_ in range(2):
        sel_np[2 * p_, p_, 0:64] = 1.0
        sel_np[2 * p_ + 1, p_, 64:128] = 1.0
    shared = {"wq": wq_t, "wk": wk_t, "wv": wv_t, "wo": wo, "w1": w1,
              "w2": w2, "fb1": fb1, "fb2": fb2, "ga": ga, "ln": ln,
              "sel": sel_np.astype(bf)}
    in_maps = []
    for b in range(B):
        xb = np.ascontiguousarray(x[b])
        in_maps.append({"x": xb, "xt": padr(np.ascontiguousarray(xb.T), DP).astype(bf),
                        **shared})

    nc = _get_nc()
    res = run_bass_kernel_spmd(nc, in_maps, core_ids=list(range(NCORES)), trace=TRACE)
    _last_results = res
    return np.stack([res.results[b]["out"] for b in range(B)], axis=0)
